# revision 66
# baseline (speedup 1.0000x reference)
"""Trainium2 Bass kernel for nn_Attention (B=8, Sq=Skv=2048, d=512).

Sharding: data-parallel over batch -- core b handles batch b (8 cores).

Per-core pipeline v3 (bf16 matmuls + XBAR DMA transposes, PE kept hot):

  The PE stream is pure matmuls: all transposes run on the DMA engines'
  hardware XBAR (InstDmaTransposeAnt, 2-byte dtypes, 14ns per 16x128
  tile), so the tensor engine never waits on vector-engine evictions and
  can hold its high p-state (the PE clock ramps 1.2 -> 2.4 GHz only
  after ~3us of gap-free execution; every stall resets it).

  stage 1 (per 128-row tile of ques/keys/vals):
    DMA in the host-pre-transposed bf16 x^T tile (packed so each DMA is
    128 contiguous 1KB descriptors) -> 4 projection matmuls (PSUM ring
    of 4 banks) -> for q/k: layernorm stats on DVE, rstd on ACT via
    exp(-0.5*ln(var+eps) + ln(gain*scale)) (the uniform LN gain and the
    1/sqrt(dk) softmax scale fold into the Exp bias; only ln/exp/copy/
    identity tables are used -> one act table load total) -> one fused
    (pr*rstd + c1) PSUM eviction to bf16 -> XBAR-transpose into
    qT/kT [d_part, d_chunk, seq].  v rows evicted to bf16 SBUF by ACT.
    LN biases are handled exactly: b_k shifts every logit row by a
    per-query constant (softmax-invariant, dropped); b_q != 0 falls back.

  stage 2 (per 128-row query tile t; causal: kv <= 128(t+1)):
    S chunks = qT.T @ kT (bf16 -> fp32 PSUM, ring 2) -> triangular mask
    on the diagonal block (+ key mask under a compile-time flag) -> exp
    on ACT with fused row-sum accumulation (no max subtraction: |S| <=
    sqrt(dk)*g^2 since q/k are layernormed) -> XBAR-transpose P chunks
    -> PV matmuls, software-pipelined one query-tile behind the S
    matmuls so the PE never waits on the P transposes -> fused
    (o/rowsum + residual) on DVE -> output layernorm -> DMA out (fp32).
"""

import math
import numpy as np

B = 8
S = 2048
D = 512
P = 128
KC = D // P       # 4 feature chunks
NT = S // P       # 16 seq tiles
EPS = 1e-5
NEG = np.float32(-1e30)

_CACHE = {}


def _bf16np():
    from concourse import mybir
    return mybir.dt.np(mybir.dt.bfloat16)


def _round_f32r(a):
    """Round fp32 to the PE's f32r grid (kept for tooling compatibility)."""
    b = np.ascontiguousarray(a, np.float32).view(np.uint32).astype(np.int64)
    low = b & 0xFFF
    base = b & ~np.int64(0xFFF)
    up = base + 0x1000
    r = np.where(low > 0x800, up,
                 np.where(low < 0x800, base,
                          np.where((base >> 12) & 1, up, base)))
    return r.astype(np.uint32).view(np.float32).reshape(a.shape)


def _build(has_km, loop_n=0, has_gobo=False):
    from contextlib import ExitStack

    import concourse.tile as tile
    from concourse import bacc, mybir

    f32 = mybir.dt.float32
    bf16 = mybir.dt.bfloat16
    fp8 = mybir.dt.float8e4
    DR = mybir.MatmulPerfMode.DoubleRow
    Alu = mybir.AluOpType
    Act = mybir.ActivationFunctionType

    class OneActSetBacc(bacc.Bacc):
        """Force every activation onto the ln+exp+copy+identity table set.

        The default chooser maps each function to the first act-func-set
        containing it (Exp -> set 0, Ln -> set 5), which makes alternating
        ln/exp insert a ~1.3us table load per pair.  This kernel only uses
        functions that all live in 'natural_log_exp_and_others', so empty
        out the earlier sets; the fixpoint pass then emits one load total.
        """

        def insert_act_table_loads(self):
            import bass_rust as _bass_rust
            from concourse.hw_specs import get_activation_tables

            has_activation = any(
                isinstance(i, mybir.InstActivation)
                for b in self.main_func.blocks
                for i in b.instructions
            )
            if not has_activation:
                return
            tables = list(get_activation_tables(self.m.arch).items())
            target = next(i for i, (n, _) in enumerate(tables)
                          if n == "natural_log_exp_and_others")
            tables = [(n, (s if i >= target else set()))
                      for i, (n, s) in enumerate(tables)]
            _bass_rust.insert_act_table_loads(self, tables)

    nc = OneActSetBacc("TRN2", target_bir_lowering=False, debug=False,
                       num_devices=B)

    # Inputs are host-packed partition-major so each tensor loads in two
    # DMAs of 128 contiguous descriptors (each HWDGE op pays ~625ns fixed
    # descriptor-generation cost, so DMA count is minimized):
    #   xT  [P, NT*D]: col (i*D + c*P + s') holds x[i*P + s', c*P + p]
    #   xq  [P, NT*D]: col (i*D + d)        holds ques[i*P + p, d]
    xq_d = nc.dram_tensor("xq", [P, NT * D], f32, kind="ExternalInput").ap()
    xqT_d = nc.dram_tensor("xqT", [P, NT * D], bf16, kind="ExternalInput").ap()
    xkT_d = nc.dram_tensor("xkT", [P, NT * D], bf16, kind="ExternalInput").ap()
    xvT_d = nc.dram_tensor("xvT", [P, NT * D], bf16, kind="ExternalInput").ap()
    wq_d = nc.dram_tensor("wq", [D, D], bf16, kind="ExternalInput").ap()
    wk_d = nc.dram_tensor("wk", [D, D], bf16, kind="ExternalInput").ap()
    wv_d = nc.dram_tensor("wv", [D, D], bf16, kind="ExternalInput").ap()
    # lnb: [:,0]=ln(g_q/sqrt(dk)), [:,1]=ln(g_k), [:,2]=ln(g_o or 1)
    lnb_d = nc.dram_tensor("lnb", [P, 3], f32, kind="ExternalInput").ap()
    gobo_d = nc.dram_tensor("gobo", [P, 2 * D], f32, kind="ExternalInput").ap()
    km_d = nc.dram_tensor("km", [P, S], f32, kind="ExternalInput").ap()
    out_d = nc.dram_tensor("out", [S, D], f32, kind="ExternalOutput").ap()

    xT_view = {"xqT": xqT_d, "xkT": xkT_d, "xvT": xvT_d}

    with tile.TileContext(nc) as tc, ExitStack() as ctx:
        cpool = ctx.enter_context(tc.tile_pool(name="consts", bufs=1))
        y_pool = ctx.enter_context(tc.tile_pool(name="ypool", bufs=4))
        small = ctx.enter_context(tc.tile_pool(name="small", bufs=8))
        p_pool = ctx.enter_context(tc.tile_pool(name="ppool", bufs=3))
        pt_pool = ctx.enter_context(tc.tile_pool(name="ptpool", bufs=8))
        z_pool = ctx.enter_context(tc.tile_pool(name="zpool", bufs=3))
        big = ctx.enter_context(tc.tile_pool(name="big", bufs=1))

        lnb = cpool.tile([P, 3], f32)
        nc.sync.dma_start(lnb[:], lnb_d)
        eps_sb = cpool.tile([P, 1], f32)
        nc.vector.memset(eps_sb[:], EPS)

        # whole-tensor input buffers, loaded in halves at the loop body
        # top (the reload overlaps the previous iteration's stage 2)
        xin = {}
        for nm in ("xqT", "xkT", "xvT"):
            xin_t = big.tile([P, NT * D], bf16, tag=nm, name=nm + "_sb")
            xin[nm] = xin_t
        xq_sb = big.tile([P, NT * D], f32, tag="xq")

        # triangular mask generated on-chip: 0 where kv <= q, -1e30 above
        tri_t = cpool.tile([P, P], f32)
        nc.gpsimd.memset(tri_t[:], 0.0)
        nc.gpsimd.affine_select(tri_t[:], tri_t[:],
                                pattern=[[-1, P]], base=0, channel_multiplier=1,
                                compare_op=mybir.AluOpType.is_ge,
                                fill=float(NEG))
        tri = tri_t[:]

        # weights (bf16, host-converted): [d_in_part, d_in_chunk, d_out]
        w_r = {}
        for name, dram in (("wq", wq_d), ("wk", wk_d), ("wv", wv_d)):
            wr = cpool.tile([P, KC, D], bf16, tag=name + "r")
            nc.sync.dma_start(wr[:], dram.rearrange("(c p) n -> p c n", p=P))
            w_r[name] = wr

        if has_gobo:
            gobo = cpool.tile([P, 2 * D], f32)
            nc.sync.dma_start(gobo[:], gobo_d)
            go_sb = gobo[:, 0:D]
            bo_sb = gobo[:, D:2 * D]
        if has_km:
            km = cpool.tile([P, S], f32)
            nc.sync.dma_start(km[:], km_d)

        # persistent per-batch tensors
        qT = big.tile([P, KC, S], bf16, tag="qT")       # [d_part, dchunk, seq]
        kT = big.tile([P, KC, S], bf16, tag="kT")
        qT8 = big.tile([P, KC, S], fp8, tag="qT8")      # fp8 copies (t>=8 QK)
        kT8 = big.tile([P, KC, S], fp8, tag="kT8")
        v_sb = big.tile([P, NT, D], bf16, tag="v")      # [kv_part, kvtile, dv]

        # identity for PE transposes (built on-chip; bf16)
        idf = cpool.tile([P, P], f32)
        nc.gpsimd.memset(idf[:], 1.0)
        nc.gpsimd.affine_select(idf[:], idf[:],
                                pattern=[[-1, P]], base=0, channel_multiplier=1,
                                compare_op=mybir.AluOpType.is_equal, fill=0.0)
        identb = cpool.tile([P, P], bf16)
        nc.vector.tensor_copy(identb[:], idf[:])
        ident = identb[:]
        expb = cpool.tile([P, 1], f32)
        nc.vector.memset(expb[:], -1.5)
        ltri_f = cpool.tile([P, P], f32)
        nc.gpsimd.memset(ltri_f[:], 1.0)
        nc.gpsimd.affine_select(ltri_f[:], ltri_f[:],
                                pattern=[[-1, P]], base=0, channel_multiplier=1,
                                compare_op=mybir.AluOpType.is_ge, fill=0.0)
        ltri01 = cpool.tile([P, P], bf16)
        nc.vector.tensor_copy(ltri01[:], ltri_f[:])

        # PSUM: proj ring 4 + transpose double-bank 1 + S ring 2 + O 1 = 8
        proj_ps = ctx.enter_context(tc.tile_pool(name="proj_ps", bufs=3,
                                                 space="PSUM"))
        tpr_ps = ctx.enter_context(tc.tile_pool(name="tpr_ps", bufs=1,
                                                space="PSUM"))
        s_ps = ctx.enter_context(tc.tile_pool(name="s_ps", bufs=2, space="PSUM"))
        o_ps = ctx.enter_context(tc.tile_pool(name="o_ps", bufs=2, space="PSUM"))
        tpr = tpr_ps.tile([P, 2 * D], bf16, tag="tpr")  # two [P,D] halves
        tpr_parity = [0]

        def tp_half():
            h = tpr_parity[0]
            tpr_parity[0] ^= 1
            return tpr[:, h * D:(h + 1) * D]

        def emit_qk_transpose(kind, i, y):
            """XBAR-transpose y [s, d] into bf16 qT/kT [d, chunk, s] (one
            DMA instruction, no PE work), then convert the tiles the fp8
            QK path reads (t>=4) on DVE/ACT.

            Query tiles t<4 read bf16 qT/kT; t>=4 read the fp8 copies.
            q tile i feeds only t=i; k tile i feeds all t>=i."""
            dstT = qT if kind == "q" else kT
            dst = dstT[:, :, i * P:(i + 1) * P]
            nc.sync.dma_start_transpose(dst, y)
            if (i >= 4) or (kind == "k"):
                dstT8 = qT8 if kind == "q" else kT8
                dst8 = dstT8[:, :, i * P:(i + 1) * P]
                if i % 2 == 1:
                    nc.scalar.copy(dst8, dst)
                else:
                    nc.vector.tensor_copy(dst8, dst)

        def proj_tile(x_ap, w, i, kind):
            # x_ap: [d_in_part, d_chunk * 128] bf16 (transposed input block)
            pr = proj_ps.tile([P, D], f32, tag="proj")
            for c in range(KC):
                nc.tensor.matmul(pr[:], x_ap[:, c * P:(c + 1) * P],
                                 w[:, c, :],
                                 start=(c == 0), stop=(c == KC - 1))
            if kind == "v":
                nc.scalar.copy(v_sb[:, i, :], pr[:])
                return None
            bn6 = small.tile([P, 6], f32, tag="bn6")
            nc.vector.bn_stats(bn6[:], pr[:])
            agg = small.tile([P, 2], f32, tag="agg")
            nc.vector.bn_aggr(agg[:], bn6[:])
            # rstd' = exp(-0.5*ln(var+eps) + ln(g*scale)) -- ln/exp tables only
            lnv = small.tile([P, 1], f32, tag="lnv")
            nc.scalar.activation(lnv[:], agg[:, 1:2], Act.Ln, bias=eps_sb[:])
            rstd = small.tile([P, 1], f32, tag="rstd")
            lnbias = lnb[:, 0:1] if kind == "q" else lnb[:, 1:2]
            nc.scalar.activation(rstd[:], lnv[:], Act.Exp, scale=-0.5,
                                 bias=lnbias)
            y = y_pool.tile([P, D], bf16, tag="y")
            if i % 2 == 0:
                c1 = small.tile([P, 1], f32, tag="c1")
                nc.vector.tensor_scalar(c1[:], agg[:, 0:1], rstd[:], -1.0,
                                        op0=Alu.mult, op1=Alu.mult)
                nc.scalar.activation(y[:], pr[:], Act.Identity,
                                     bias=c1[:], scale=rstd[:])
            else:
                nc.vector.tensor_scalar(y[:], pr[:], agg[:, 0:1], rstd[:],
                                        op0=Alu.subtract, op1=Alu.mult)
            return (kind, i, y)

        def s_block(t):
            """QK matmuls, exp, diagonal-triangle zeroing for query tile t.

            No additive mask on the diagonal block: exp runs unmasked (values
            stay finite: |S| <= sqrt(dk)*g^2), then the strictly-upper
            triangle of P's diagonal 128x128 block is zeroed on gpsimd and
            its row-sum contribution computed separately, keeping the
            S-chunk PSUM critical chain at QK->exp only.
            """
            L = P * (t + 1)
            n_chunks = (L + 511) // 512
            p_sb = p_pool.tile([P, S], bf16, tag="p")
            sums = small.tile([P, KC + 1], f32, tag="sums")
            pts = []
            for c in range(n_chunks):
                w_cols = min(512, L - c * 512)
                sc = s_ps.tile([P, 512], f32, tag="s")
                if t >= 4:
                    for u in range(2):
                        nc.tensor.matmul(sc[:, :w_cols],
                                         qT8[:, 2 * u:2 * u + 2,
                                             t * P:(t + 1) * P],
                                         kT8[:, 2 * u:2 * u + 2,
                                             c * 512:c * 512 + w_cols],
                                         start=(u == 0), stop=(u == 1),
                                         perf_mode=DR)
                else:
                    for kc in range(KC):
                        nc.tensor.matmul(sc[:, :w_cols],
                                         qT[:, kc, t * P:(t + 1) * P],
                                         kT[:, kc, c * 512:c * 512 + w_cols],
                                         start=(kc == 0),
                                         stop=(kc == KC - 1))
                if has_km:
                    nc.vector.tensor_tensor(sc[:, :w_cols], sc[:, :w_cols],
                                            km[:, c * 512:c * 512 + w_cols],
                                            op=Alu.add)
                diag = c * 512 <= t * P < c * 512 + w_cols
                if diag:
                    off = t * P - c * 512
                    if off > 0:
                        nc.scalar.activation(p_sb[:, c * 512:c * 512 + off],
                                             sc[:, :off], Act.Exp,
                                             bias=expb[:],
                                             accum_out=sums[:, c:c + 1])
                    else:
                        nc.vector.memset(sums[:, c:c + 1], 0.0)
                    nc.scalar.activation(p_sb[:, t * P:(t + 1) * P],
                                         sc[:, off:off + P], Act.Exp,
                                         bias=expb[:])
                    # zero strictly-upper triangle, then add its row sums
                    nc.vector.tensor_tensor(p_sb[:, t * P:(t + 1) * P],
                                            p_sb[:, t * P:(t + 1) * P],
                                            ltri01[:], op=Alu.mult)
                    nc.vector.tensor_reduce(sums[:, n_chunks:n_chunks + 1],
                                            p_sb[:, t * P:(t + 1) * P],
                                            axis=mybir.AxisListType.X,
                                            op=Alu.add)
                else:
                    nc.scalar.activation(p_sb[:, c * 512:c * 512 + w_cols],
                                         sc[:, :w_cols], Act.Exp,
                                         bias=expb[:],
                                         accum_out=sums[:, c:c + 1])
            return p_sb, sums, n_chunks

        def pv_block(t, state):
            """P transposes + PV matmuls + output chain for tile t (lagged).

            Transposes lead the PV matmuls by two batches so the PSUM
            eviction of batch b is off the PE's critical path.
            """
            p_sb, sums, n_chunks = state
            rr = small.tile([P, 1], f32, tag="rr")
            ssum = small.tile([P, 1], f32, tag="ssum")
            nc.vector.tensor_reduce(ssum[:], sums[:, :n_chunks + 1],
                                    axis=mybir.AxisListType.X, op=Alu.add)
            nc.vector.reciprocal(rr[:], ssum[:])

            nb = t + 1
            batches = [(b, min(4, nb - b * 4)) for b in range((nb + 3) // 4)]
            pt_sbs = {}

            def emit_tp(b, jn):
                half = tp_half()
                for j in range(jn):
                    jj = b * 4 + j
                    nc.tensor.transpose(half[:, j * P:(j + 1) * P],
                                        p_sb[:, jj * P:(jj + 1) * P], ident)
                pt_sb = pt_pool.tile([P, D], bf16, tag="pt")
                nc.vector.tensor_copy(pt_sb[:, :jn * P], half[:, :jn * P])
                pt_sbs[b] = pt_sb

            ops = o_ps.tile([P, D], f32, tag="o")

            def emit_pv(b, jn):
                pt_sb = pt_sbs.pop(b)
                for j in range(jn):
                    jj = b * 4 + j
                    nc.tensor.matmul(ops[:], pt_sb[:, j * P:(j + 1) * P],
                                     v_sb[:, jj, :],
                                     start=(jj == 0), stop=(jj == t))

            for b, jn in batches:
                if b >= 2:
                    emit_pv(b - 2, batches[b - 2][1])
                emit_tp(b, jn)
            for b, jn in batches[-2:]:
                emit_pv(b, jn)

            # out = LN(o / rowsum + xq) [* go + bo]
            xres = xq_sb[:, t * D:(t + 1) * D]
            z = z_pool.tile([P, D], f32, tag="z")
            nc.vector.scalar_tensor_tensor(z[:], ops[:], rr[:], xres,
                                           op0=Alu.mult, op1=Alu.add)
            bn6 = small.tile([P, 6], f32, tag="bn6z")
            nc.vector.bn_stats(bn6[:], z[:])
            agg = small.tile([P, 2], f32, tag="aggz")
            nc.vector.bn_aggr(agg[:], bn6[:])
            lnv = small.tile([P, 1], f32, tag="lnvz")
            nc.scalar.activation(lnv[:], agg[:, 1:2], Act.Ln, bias=eps_sb[:])
            rstd = small.tile([P, 1], f32, tag="rstdz")
            nc.scalar.activation(rstd[:], lnv[:], Act.Exp, scale=-0.5,
                                 bias=lnb[:, 2:3])
            w1 = z_pool.tile([P, D], f32, tag="w1")
            if t % 2 == 0:
                c1 = small.tile([P, 1], f32, tag="c1z")
                nc.vector.tensor_scalar(c1[:], agg[:, 0:1], rstd[:], -1.0,
                                        op0=Alu.mult, op1=Alu.mult)
                nc.scalar.activation(w1[:], z[:], Act.Identity,
                                     bias=c1[:], scale=rstd[:])
            else:
                nc.vector.tensor_scalar(w1[:], z[:], agg[:, 0:1], rstd[:],
                                        op0=Alu.subtract, op1=Alu.mult)
            if has_gobo:
                o_sb = z_pool.tile([P, D], f32, tag="osb")
                nc.vector.tensor_tensor(o_sb[:], w1[:], go_sb, op=Alu.mult)
                nc.vector.tensor_tensor(o_sb[:], o_sb[:], bo_sb, op=Alu.add)
                nc.scalar.dma_start(out_d[t * P:(t + 1) * P, :], o_sb[:])
            else:
                nc.scalar.dma_start(out_d[t * P:(t + 1) * P, :], w1[:])

        loop_cm = tc.For_i(0, loop_n, 1) if loop_n else None
        if loop_cm is not None:
            loop_cm.__enter__()
        H = NT * D // 2
        for nm in ("xqT", "xkT", "xvT"):
            for h in range(2):
                nc.sync.dma_start(xin[nm][:, h * H:(h + 1) * H],
                                  xT_view[nm][:, h * H:(h + 1) * H])
        for h in range(2):
            nc.sync.dma_start(xq_sb[:, h * H:(h + 1) * H],
                              xq_d[:, h * H:(h + 1) * H])
        # ---- stage 1 (q/k transposes lag their projections by ~3 groups
        #      so the LN chain is never on the PE's critical path) ----
        pending = []
        for i in range(NT):
            for nm, wkey, kind in (("xqT", "wq", "q"),
                                   ("xkT", "wk", "k"),
                                   ("xvT", "wv", "v")):
                res = proj_tile(xin[nm][:, i * D:(i + 1) * D],
                                w_r[wkey], i, kind)
                if res is not None:
                    pending.append(res)
                while len(pending) > 2:
                    k2, i2, y2 = pending.pop(0)
                    emit_qk_transpose(k2, i2, y2)
        for k2, i2, y2 in pending:
            emit_qk_transpose(k2, i2, y2)
        pending = []
        # ---- stage 2 (PV lags S by one query tile) ----
        prev = None
        for t in range(NT):
            state = s_block(t)
            if prev is not None:
                pv_block(t - 1, prev)
            prev = state
        pv_block(NT - 1, prev)
        if loop_cm is not None:
            loop_cm.__exit__(None, None, None)

    nc.compile()
    return nc


def _get_nc(has_km=False, has_gobo=False):
    key = ("nc", bool(has_km), bool(has_gobo))
    if key not in _CACHE:
        _CACHE[key] = _build(has_km, has_gobo=has_gobo)
    return _CACHE[key]


def _bench_inputs(rng):
    """Input map (one core) with the same shapes/dtypes kernel() feeds."""
    f = np.float32
    bf = _bf16np()
    xq = rng.standard_normal((S, D), dtype=f)
    pack = lambda x: np.ascontiguousarray(
        x.reshape(NT, P, KC, P).transpose(3, 0, 2, 1)).reshape(P, NT * D).astype(bf)
    mkT = lambda: pack(rng.standard_normal((S, D), dtype=f))
    lnb = np.zeros((P, 3), f)
    lnb[:, 0] = -0.5 * math.log(D)
    xqp = np.ascontiguousarray(
        xq.reshape(NT, P, D).transpose(1, 0, 2)).reshape(P, NT * D)
    return {
        "xq": xqp, "xqT": pack(xq),
        "xkT": mkT(), "xvT": mkT(),
        "wq": rng.standard_normal((D, D), dtype=f).astype(bf),
        "wk": rng.standard_normal((D, D), dtype=f).astype(bf),
        "wv": rng.standard_normal((D, D), dtype=f).astype(bf),
        "lnb": lnb, "gobo": np.ones((P, 2 * D), f),
        "km": np.zeros((P, S), f),
    }


def _fallback(vals, keys, ques, causal_mask, key_mask, Wv, Wk, Wq,
              ln_k_g, ln_k_b, ln_q_g, ln_q_b, ln_o_g, ln_o_b):
    # numpy reference path; used when the inputs fall outside the pattern
    # this kernel is specialized for.
    def ln(x, g, b):
        mu = x.mean(-1, keepdims=True)
        var = ((x - mu) ** 2).mean(-1, keepdims=True)
        return (x - mu) / np.sqrt(var + EPS) * g + b

    x64 = np.float64
    vals, keys, ques = (np.asarray(a) for a in (vals, keys, ques))
    v = vals.astype(x64) @ np.asarray(Wv, x64)
    k = ln(keys.astype(x64) @ np.asarray(Wk, x64), np.asarray(ln_k_g),
           np.asarray(ln_k_b))
    q = ln(ques.astype(x64) @ np.asarray(Wq, x64), np.asarray(ln_q_g),
           np.asarray(ln_q_b))
    a = np.einsum("bqd,bkd->bqk", q, k) / math.sqrt(D)
    a = np.where(causal_mask[None], -np.inf, a)
    a = np.where(key_mask[:, None, :], -np.inf, a)
    a = a - a.max(-1, keepdims=True)
    p = np.exp(a)
    p /= p.sum(-1, keepdims=True)
    o = np.einsum("bqk,bkd->bqd", p, v)
    return np.asarray(ln(o + ques.astype(x64), np.asarray(ln_o_g),
                         np.asarray(ln_o_b)), np.float32)


def _get_runner(has_km, has_gobo):
    """Build (once) a cached sharded-jit executor for the compiled module."""
    key = ("runner", bool(has_km), bool(has_gobo))
    if key in _CACHE:
        return _CACHE[key]

    import jax
    import numpy as _np
    from jax.sharding import Mesh, PartitionSpec
    from jax.experimental.shard_map import shard_map
    from concourse import mybir
    from concourse.bass2jax import (_bass_exec_p, install_neuronx_cc_hook,
                                    partition_id_tensor)

    install_neuronx_cc_hook()
    nc = _get_nc(has_km, has_gobo)

    pname = nc.partition_id_tensor.name if nc.partition_id_tensor else None
    in_names, out_names, out_avals, zero_outs = [], [], [], []
    for alloc in nc.m.functions[0].allocations:
        if not isinstance(alloc, mybir.MemoryLocationSet):
            continue
        name = alloc.memorylocations[0].name
        if alloc.kind == "ExternalInput":
            if name != pname:
                in_names.append(name)
        elif alloc.kind == "ExternalOutput":
            shape = tuple(alloc.tensor_shape)
            dtype = mybir.dt.np(alloc.dtype)
            out_names.append(name)
            out_avals.append(jax.core.ShapedArray(shape, dtype))
            zero_outs.append(_np.zeros((B * shape[0], *shape[1:]), dtype))
    n_params = len(in_names)
    all_in = in_names + out_names
    if pname is not None:
        all_in = all_in + [pname]

    def _body(*args):
        operands = list(args)
        if pname is not None:
            operands.append(partition_id_tensor())
        outs = _bass_exec_p.bind(
            *operands,
            out_avals=tuple(out_avals),
            in_names=tuple(all_in),
            out_names=tuple(out_names),
            lowering_input_output_aliases=(),
            sim_require_finite=True,
            sim_require_nnan=True,
            nc=nc,
        )
        return tuple(outs)

    devices = jax.devices()[:B]
    mesh = Mesh(np.asarray(devices), ("core",))
    donate = tuple(range(n_params, n_params + len(out_names)))
    sharded = jax.jit(
        shard_map(_body, mesh=mesh,
                  in_specs=(PartitionSpec("core"),) * (n_params + len(out_names)),
                  out_specs=(PartitionSpec("core"),) * len(out_names),
                  check_rep=False),
        donate_argnums=donate, keep_unused=True)

    def run(concat_by_name):
        args = [concat_by_name[n] for n in in_names] + list(zero_outs)
        out_arrs = sharded(*args)
        return {n: _np.asarray(out_arrs[i]).reshape(B, *out_avals[i].shape)
                for i, n in enumerate(out_names)}

    _CACHE[key] = run
    return run


def _uniform_pos(v):
    v = np.asarray(v, np.float32)
    return v.size > 0 and np.all(v == v.flat[0]) and v.flat[0] > 0


def kernel(vals, keys, ques, causal_mask, key_mask, Wv, Wk, Wq,
           ln_k_g, ln_k_b, ln_q_g, ln_q_b, ln_o_g, ln_o_b):
    causal_mask = np.asarray(causal_mask)
    key_mask = np.asarray(key_mask)
    ln_q_b = np.asarray(ln_q_b, np.float32)
    ln_k_b = np.asarray(ln_k_b, np.float32)
    ln_o_g = np.asarray(ln_o_g, np.float32)
    ln_o_b = np.asarray(ln_o_b, np.float32)
    # The device kernel is specialized for: standard causal triu mask,
    # b_q == 0 (b_k is dropped exactly -- it shifts each logit row by a
    # per-query constant, which softmax ignores), uniform positive q/k
    # gains (folded into the Exp bias producing rstd).
    ok = (np.array_equal(causal_mask, np.triu(np.ones((S, S), bool), k=1))
          and not ln_q_b.any()
          and float(np.abs(ln_k_b).max(initial=0.0)) < 16.0
          and _uniform_pos(ln_q_g) and _uniform_pos(ln_k_g))
    if not ok:
        return _fallback(vals, keys, ques, causal_mask, key_mask, Wv, Wk, Wq,
                         ln_k_g, ln_k_b, ln_q_g, ln_q_b, ln_o_g, ln_o_b)

    has_km = bool(key_mask.any())
    has_gobo = not (_uniform_pos(ln_o_g) and not ln_o_b.any())
    run = _get_runner(has_km, has_gobo)

    f = np.float32
    bf = _bf16np()

    lnb = np.zeros((P, 3), f)
    lnb[:, 0] = math.log(float(np.asarray(ln_q_g).flat[0]) / math.sqrt(D))
    lnb[:, 1] = math.log(float(np.asarray(ln_k_g).flat[0]))
    lnb[:, 2] = 0.0 if has_gobo else math.log(float(ln_o_g.flat[0]))
    gobo = np.broadcast_to(
        np.concatenate([ln_o_g, ln_o_b]), (P, 2 * D)).copy()
    xq = np.ascontiguousarray(
        np.asarray(ques, f).reshape(B, NT, P, D).transpose(0, 2, 1, 3)
    ).reshape(B * P, NT * D)

    def xT(a):
        # [B, S, D] fp32 -> per-batch partition-major packed transpose,
        # bf16 [B*P, NT*D]: col (i*D + c*P + s') = a[b, i*P+s', c*P+p]
        return np.ascontiguousarray(
            np.asarray(a, f).reshape(B, NT, P, KC, P).transpose(0, 4, 1, 3, 2)
        ).reshape(B * P, NT * D).astype(bf)

    def rep(a):
        return np.concatenate([a] * B, axis=0)

    km_rows = np.where(key_mask, NEG, f(0)).astype(f)          # [B, S]
    km_cat = np.repeat(km_rows, P, axis=0)                      # [B*P, S]
    concat = {
        "xq": xq,
        "xqT": xT(ques), "xkT": xT(keys), "xvT": xT(vals),
        "wq": rep(np.ascontiguousarray(Wq, f).astype(bf)),
        "wk": rep(np.ascontiguousarray(Wk, f).astype(bf)),
        "wv": rep(np.ascontiguousarray(Wv, f).astype(bf)),
        "lnb": rep(lnb), "gobo": rep(gobo),
        "km": km_cat,
    }
    out = run(concat)["out"]                                    # [B, S, D]
    return out



# revision 67
# speedup vs baseline: 1.0956x; 1.0956x over previous
"""Trainium2 Bass kernel for nn_Attention (B=8, Sq=Skv=2048, d=512).

Sharding: data-parallel over batch -- core b handles batch b (8 cores).

Per-core pipeline v3 (bf16 matmuls + XBAR DMA transposes, PE kept hot):

  The PE stream is pure matmuls: all transposes run on the DMA engines'
  hardware XBAR (InstDmaTransposeAnt, 2-byte dtypes, 14ns per 16x128
  tile), so the tensor engine never waits on vector-engine evictions and
  can hold its high p-state (the PE clock ramps 1.2 -> 2.4 GHz only
  after ~3us of gap-free execution; every stall resets it).

  stage 1 (per 128-row tile of ques/keys/vals):
    DMA in the host-pre-transposed bf16 x^T tile (packed so each DMA is
    128 contiguous 1KB descriptors) -> 4 projection matmuls (PSUM ring
    of 4 banks) -> for q/k: layernorm stats on DVE, rstd on ACT via
    exp(-0.5*ln(var+eps) + ln(gain*scale)) (the uniform LN gain and the
    1/sqrt(dk) softmax scale fold into the Exp bias; only ln/exp/copy/
    identity tables are used -> one act table load total) -> one fused
    (pr*rstd + c1) PSUM eviction to bf16 -> XBAR-transpose into
    qT/kT [d_part, d_chunk, seq].  v rows evicted to bf16 SBUF by ACT.
    LN biases are handled exactly: b_k shifts every logit row by a
    per-query constant (softmax-invariant, dropped); b_q != 0 falls back.

  stage 2 (per 128-row query tile t; causal: kv <= 128(t+1)):
    S chunks = qT.T @ kT (bf16 -> fp32 PSUM, ring 2) -> triangular mask
    on the diagonal block (+ key mask under a compile-time flag) -> exp
    on ACT with fused row-sum accumulation (no max subtraction: |S| <=
    sqrt(dk)*g^2 since q/k are layernormed) -> XBAR-transpose P chunks
    -> PV matmuls, software-pipelined one query-tile behind the S
    matmuls so the PE never waits on the P transposes -> fused
    (o/rowsum + residual) on DVE -> output layernorm -> DMA out (fp32).
"""

import math
import numpy as np

B = 8
S = 2048
D = 512
P = 128
KC = D // P       # 4 feature chunks
NT = S // P       # 16 seq tiles
EPS = 1e-5
NEG = np.float32(-1e30)

_CACHE = {}


def _bf16np():
    from concourse import mybir
    return mybir.dt.np(mybir.dt.bfloat16)


def _round_f32r(a):
    """Round fp32 to the PE's f32r grid (kept for tooling compatibility)."""
    b = np.ascontiguousarray(a, np.float32).view(np.uint32).astype(np.int64)
    low = b & 0xFFF
    base = b & ~np.int64(0xFFF)
    up = base + 0x1000
    r = np.where(low > 0x800, up,
                 np.where(low < 0x800, base,
                          np.where((base >> 12) & 1, up, base)))
    return r.astype(np.uint32).view(np.float32).reshape(a.shape)


def _build(has_km, loop_n=0, has_gobo=False):
    from contextlib import ExitStack

    import concourse.tile as tile
    from concourse import bacc, mybir

    f32 = mybir.dt.float32
    bf16 = mybir.dt.bfloat16
    fp8 = mybir.dt.float8e4
    DR = mybir.MatmulPerfMode.DoubleRow
    Alu = mybir.AluOpType
    Act = mybir.ActivationFunctionType

    class OneActSetBacc(bacc.Bacc):
        """Force every activation onto the ln+exp+copy+identity table set.

        The default chooser maps each function to the first act-func-set
        containing it (Exp -> set 0, Ln -> set 5), which makes alternating
        ln/exp insert a ~1.3us table load per pair.  This kernel only uses
        functions that all live in 'natural_log_exp_and_others', so empty
        out the earlier sets; the fixpoint pass then emits one load total.
        """

        def insert_act_table_loads(self):
            import bass_rust as _bass_rust
            from concourse.hw_specs import get_activation_tables

            has_activation = any(
                isinstance(i, mybir.InstActivation)
                for b in self.main_func.blocks
                for i in b.instructions
            )
            if not has_activation:
                return
            tables = list(get_activation_tables(self.m.arch).items())
            target = next(i for i, (n, _) in enumerate(tables)
                          if n == "natural_log_exp_and_others")
            tables = [(n, (s if i >= target else set()))
                      for i, (n, s) in enumerate(tables)]
            _bass_rust.insert_act_table_loads(self, tables)

    nc = OneActSetBacc("TRN2", target_bir_lowering=False, debug=False,
                       num_devices=B)

    # xT tensors are host-packed tile-major: row (i*P + p), col (c*P + s')
    # holds x[i*P + s', c*P + p] -- each per-tile DMA is 128 contiguous 1KB
    # descriptors (smaller descriptors are descriptor-rate bound on HWDGE).
    xq_d = nc.dram_tensor("xq", [S, D], f32, kind="ExternalInput").ap()
    xqT_d = nc.dram_tensor("xqT", [S, D], bf16, kind="ExternalInput").ap()
    xkT_d = nc.dram_tensor("xkT", [S, D], bf16, kind="ExternalInput").ap()
    xvT_d = nc.dram_tensor("xvT", [S, D], bf16, kind="ExternalInput").ap()
    wq_d = nc.dram_tensor("wq", [D, D], bf16, kind="ExternalInput").ap()
    wk_d = nc.dram_tensor("wk", [D, D], bf16, kind="ExternalInput").ap()
    wv_d = nc.dram_tensor("wv", [D, D], bf16, kind="ExternalInput").ap()
    # lnb: [:,0]=ln(g_q/sqrt(dk)), [:,1]=ln(g_k), [:,2]=ln(g_o or 1)
    lnb_d = nc.dram_tensor("lnb", [P, 3], f32, kind="ExternalInput").ap()
    gobo_d = nc.dram_tensor("gobo", [P, 2 * D], f32, kind="ExternalInput").ap()
    km_d = nc.dram_tensor("km", [P, S], f32, kind="ExternalInput").ap()
    out_d = nc.dram_tensor("out", [S, D], f32, kind="ExternalOutput").ap()

    xT_view = {"xqT": xqT_d, "xkT": xkT_d, "xvT": xvT_d}

    with tile.TileContext(nc) as tc, ExitStack() as ctx:
        cpool = ctx.enter_context(tc.tile_pool(name="consts", bufs=1))
        xstage = ctx.enter_context(tc.tile_pool(name="xstage", bufs=6))
        y_pool = ctx.enter_context(tc.tile_pool(name="ypool", bufs=4))
        small = ctx.enter_context(tc.tile_pool(name="small", bufs=8))
        p_pool = ctx.enter_context(tc.tile_pool(name="ppool", bufs=3))
        pt_pool = ctx.enter_context(tc.tile_pool(name="ptpool", bufs=8))
        z_pool = ctx.enter_context(tc.tile_pool(name="zpool", bufs=3))
        big = ctx.enter_context(tc.tile_pool(name="big", bufs=1))

        lnb = cpool.tile([P, 3], f32)
        nc.sync.dma_start(lnb[:], lnb_d)
        eps_sb = cpool.tile([P, 1], f32)
        nc.vector.memset(eps_sb[:], EPS)

        # prefetch the first input tiles (single-shot mode only; in the
        # loop the DMAs at the body top naturally prefetch across passes)
        prefetched = {}
        for i in () if loop_n else (0, 1):
            for nm in ("xqT", "xkT", "xvT"):
                xt0 = xstage.tile([P, KC * P], bf16, tag=nm)
                nc.sync.dma_start(xt0[:], xT_view[nm][i * P:(i + 1) * P, :])
                prefetched[(nm, i)] = xt0

        # triangular mask generated on-chip: 0 where kv <= q, -1e30 above
        tri_t = cpool.tile([P, P], f32)
        nc.gpsimd.memset(tri_t[:], 0.0)
        nc.gpsimd.affine_select(tri_t[:], tri_t[:],
                                pattern=[[-1, P]], base=0, channel_multiplier=1,
                                compare_op=mybir.AluOpType.is_ge,
                                fill=float(NEG))
        tri = tri_t[:]

        # weights (bf16, host-converted): [d_in_part, d_in_chunk, d_out]
        w_r = {}
        for name, dram in (("wq", wq_d), ("wk", wk_d), ("wv", wv_d)):
            wr = cpool.tile([P, KC, D], bf16, tag=name + "r")
            nc.sync.dma_start(wr[:], dram.rearrange("(c p) n -> p c n", p=P))
            w_r[name] = wr

        if has_gobo:
            gobo = cpool.tile([P, 2 * D], f32)
            nc.sync.dma_start(gobo[:], gobo_d)
            go_sb = gobo[:, 0:D]
            bo_sb = gobo[:, D:2 * D]
        if has_km:
            km = cpool.tile([P, S], f32)
            nc.sync.dma_start(km[:], km_d)

        # persistent per-batch tensors
        qT = big.tile([P, KC, S], bf16, tag="qT")       # [d_part, dchunk, seq]
        kT = big.tile([P, KC, S], bf16, tag="kT")
        qT8 = big.tile([P, KC, S], fp8, tag="qT8")      # fp8 copies (t>=8 QK)
        kT8 = big.tile([P, KC, S], fp8, tag="kT8")
        v_sb = big.tile([P, NT, D], bf16, tag="v")      # [kv_part, kvtile, dv]

        # identity for PE transposes (built on-chip; bf16)
        idf = cpool.tile([P, P], f32)
        nc.gpsimd.memset(idf[:], 1.0)
        nc.gpsimd.affine_select(idf[:], idf[:],
                                pattern=[[-1, P]], base=0, channel_multiplier=1,
                                compare_op=mybir.AluOpType.is_equal, fill=0.0)
        identb = cpool.tile([P, P], bf16)
        nc.vector.tensor_copy(identb[:], idf[:])
        ident = identb[:]
        expb = cpool.tile([P, 1], f32)
        nc.vector.memset(expb[:], -1.5)
        ltri_f = cpool.tile([P, P], f32)
        nc.gpsimd.memset(ltri_f[:], 1.0)
        nc.gpsimd.affine_select(ltri_f[:], ltri_f[:],
                                pattern=[[-1, P]], base=0, channel_multiplier=1,
                                compare_op=mybir.AluOpType.is_ge, fill=0.0)
        ltri01 = cpool.tile([P, P], bf16)
        nc.vector.tensor_copy(ltri01[:], ltri_f[:])

        # PSUM: proj ring 4 + transpose double-bank 1 + S ring 2 + O 1 = 8
        proj_ps = ctx.enter_context(tc.tile_pool(name="proj_ps", bufs=3,
                                                 space="PSUM"))
        tpr_ps = ctx.enter_context(tc.tile_pool(name="tpr_ps", bufs=1,
                                                space="PSUM"))
        s_ps = ctx.enter_context(tc.tile_pool(name="s_ps", bufs=2, space="PSUM"))
        o_ps = ctx.enter_context(tc.tile_pool(name="o_ps", bufs=2, space="PSUM"))
        tpr = tpr_ps.tile([P, 2 * D], bf16, tag="tpr")  # two [P,D] halves
        tpr_parity = [0]

        def tp_half():
            h = tpr_parity[0]
            tpr_parity[0] ^= 1
            return tpr[:, h * D:(h + 1) * D]

        def emit_qk_transpose(kind, i, y, half):
            """PE-transpose y -> [d, seq]; evict to the dtype(s) needed.

            Query tiles t<8 read bf16 qT/kT; t>=8 read the fp8 copies.
            q tile i feeds only t=i; k tile i feeds all t>=i."""
            for c in range(KC):
                nc.tensor.transpose(half[:, c * P:(c + 1) * P],
                                    y[:, c * P:(c + 1) * P], ident)
            src = half.rearrange("p (c s) -> p c s", c=KC)
            want_bf = (i < 4)
            want_f8 = (i >= 4) or (kind == "k")
            if want_bf:
                dstT = qT if kind == "q" else kT
                dst = dstT[:, :, i * P:(i + 1) * P]
                if i % 2 == 0:
                    nc.scalar.copy(dst, src)
                else:
                    nc.vector.tensor_copy(dst, src)
            if want_f8:
                dstT8 = qT8 if kind == "q" else kT8
                dst8 = dstT8[:, :, i * P:(i + 1) * P]
                if i % 2 == 1:
                    nc.scalar.copy(dst8, src)
                else:
                    nc.vector.tensor_copy(dst8, src)

        def proj_tile(x_ap, w, i, kind):
            # x_ap: [d_in_part, d_chunk * 128] bf16 (transposed input block)
            pr = proj_ps.tile([P, D], f32, tag="proj")
            for c in range(KC):
                nc.tensor.matmul(pr[:], x_ap[:, c * P:(c + 1) * P],
                                 w[:, c, :],
                                 start=(c == 0), stop=(c == KC - 1))
            if kind == "v":
                nc.scalar.copy(v_sb[:, i, :], pr[:])
                return None
            bn6 = small.tile([P, 6], f32, tag="bn6")
            nc.vector.bn_stats(bn6[:], pr[:])
            agg = small.tile([P, 2], f32, tag="agg")
            nc.vector.bn_aggr(agg[:], bn6[:])
            # rstd' = exp(-0.5*ln(var+eps) + ln(g*scale)) -- ln/exp tables only
            lnv = small.tile([P, 1], f32, tag="lnv")
            nc.scalar.activation(lnv[:], agg[:, 1:2], Act.Ln, bias=eps_sb[:])
            rstd = small.tile([P, 1], f32, tag="rstd")
            lnbias = lnb[:, 0:1] if kind == "q" else lnb[:, 1:2]
            nc.scalar.activation(rstd[:], lnv[:], Act.Exp, scale=-0.5,
                                 bias=lnbias)
            y = y_pool.tile([P, D], bf16, tag="y")
            if i % 2 == 0:
                c1 = small.tile([P, 1], f32, tag="c1")
                nc.vector.tensor_scalar(c1[:], agg[:, 0:1], rstd[:], -1.0,
                                        op0=Alu.mult, op1=Alu.mult)
                nc.scalar.activation(y[:], pr[:], Act.Identity,
                                     bias=c1[:], scale=rstd[:])
            else:
                nc.vector.tensor_scalar(y[:], pr[:], agg[:, 0:1], rstd[:],
                                        op0=Alu.subtract, op1=Alu.mult)
            return (kind, i, y)

        def s_block(t):
            """QK matmuls, exp, diagonal-triangle zeroing for query tile t.

            No additive mask on the diagonal block: exp runs unmasked (values
            stay finite: |S| <= sqrt(dk)*g^2), then the strictly-upper
            triangle of P's diagonal 128x128 block is zeroed on gpsimd and
            its row-sum contribution computed separately, keeping the
            S-chunk PSUM critical chain at QK->exp only.
            """
            L = P * (t + 1)
            n_chunks = (L + 511) // 512
            p_sb = p_pool.tile([P, S], bf16, tag="p")
            sums = small.tile([P, KC + 1], f32, tag="sums")
            pts = []
            for c in range(n_chunks):
                w_cols = min(512, L - c * 512)
                sc = s_ps.tile([P, 512], f32, tag="s")
                if t >= 4:
                    for u in range(2):
                        nc.tensor.matmul(sc[:, :w_cols],
                                         qT8[:, 2 * u:2 * u + 2,
                                             t * P:(t + 1) * P],
                                         kT8[:, 2 * u:2 * u + 2,
                                             c * 512:c * 512 + w_cols],
                                         start=(u == 0), stop=(u == 1),
                                         perf_mode=DR)
                else:
                    for kc in range(KC):
                        nc.tensor.matmul(sc[:, :w_cols],
                                         qT[:, kc, t * P:(t + 1) * P],
                                         kT[:, kc, c * 512:c * 512 + w_cols],
                                         start=(kc == 0),
                                         stop=(kc == KC - 1))
                if has_km:
                    nc.vector.tensor_tensor(sc[:, :w_cols], sc[:, :w_cols],
                                            km[:, c * 512:c * 512 + w_cols],
                                            op=Alu.add)
                diag = c * 512 <= t * P < c * 512 + w_cols
                if diag:
                    off = t * P - c * 512
                    if off > 0:
                        nc.scalar.activation(p_sb[:, c * 512:c * 512 + off],
                                             sc[:, :off], Act.Exp,
                                             bias=expb[:],
                                             accum_out=sums[:, c:c + 1])
                    else:
                        nc.vector.memset(sums[:, c:c + 1], 0.0)
                    nc.scalar.activation(p_sb[:, t * P:(t + 1) * P],
                                         sc[:, off:off + P], Act.Exp,
                                         bias=expb[:])
                    # zero strictly-upper triangle, then add its row sums
                    nc.vector.tensor_tensor(p_sb[:, t * P:(t + 1) * P],
                                            p_sb[:, t * P:(t + 1) * P],
                                            ltri01[:], op=Alu.mult)
                    nc.vector.tensor_reduce(sums[:, n_chunks:n_chunks + 1],
                                            p_sb[:, t * P:(t + 1) * P],
                                            axis=mybir.AxisListType.X,
                                            op=Alu.add)
                else:
                    nc.scalar.activation(p_sb[:, c * 512:c * 512 + w_cols],
                                         sc[:, :w_cols], Act.Exp,
                                         bias=expb[:],
                                         accum_out=sums[:, c:c + 1])
            return p_sb, sums, n_chunks

        def pv_block(t, state):
            """P transposes + PV matmuls + output chain for tile t (lagged).

            Transposes lead the PV matmuls by two batches so the PSUM
            eviction of batch b is off the PE's critical path.
            """
            p_sb, sums, n_chunks = state
            rr = small.tile([P, 1], f32, tag="rr")
            ssum = small.tile([P, 1], f32, tag="ssum")
            nc.vector.tensor_reduce(ssum[:], sums[:, :n_chunks + 1],
                                    axis=mybir.AxisListType.X, op=Alu.add)
            nc.vector.reciprocal(rr[:], ssum[:])

            nb = t + 1
            batches = [(b, min(4, nb - b * 4)) for b in range((nb + 3) // 4)]
            pt_sbs = {}

            def emit_tp(b, jn):
                half = tp_half()
                for j in range(jn):
                    jj = b * 4 + j
                    nc.tensor.transpose(half[:, j * P:(j + 1) * P],
                                        p_sb[:, jj * P:(jj + 1) * P], ident)
                pt_sb = pt_pool.tile([P, D], bf16, tag="pt")
                nc.vector.tensor_copy(pt_sb[:, :jn * P], half[:, :jn * P])
                pt_sbs[b] = pt_sb

            ops = o_ps.tile([P, D], f32, tag="o")

            def emit_pv(b, jn):
                pt_sb = pt_sbs.pop(b)
                for j in range(jn):
                    jj = b * 4 + j
                    nc.tensor.matmul(ops[:], pt_sb[:, j * P:(j + 1) * P],
                                     v_sb[:, jj, :],
                                     start=(jj == 0), stop=(jj == t))

            for b, jn in batches:
                if b >= 2:
                    emit_pv(b - 2, batches[b - 2][1])
                emit_tp(b, jn)
            for b, jn in batches[-2:]:
                emit_pv(b, jn)

            # out = LN(o / rowsum + xq) [* go + bo]
            xres = z_pool.tile([P, D], f32, tag="xres")
            nc.sync.dma_start(xres[:], xq_d[t * P:(t + 1) * P, :])
            z = z_pool.tile([P, D], f32, tag="z")
            nc.vector.scalar_tensor_tensor(z[:], ops[:], rr[:], xres[:],
                                           op0=Alu.mult, op1=Alu.add)
            bn6 = small.tile([P, 6], f32, tag="bn6z")
            nc.vector.bn_stats(bn6[:], z[:])
            agg = small.tile([P, 2], f32, tag="aggz")
            nc.vector.bn_aggr(agg[:], bn6[:])
            lnv = small.tile([P, 1], f32, tag="lnvz")
            nc.scalar.activation(lnv[:], agg[:, 1:2], Act.Ln, bias=eps_sb[:])
            rstd = small.tile([P, 1], f32, tag="rstdz")
            nc.scalar.activation(rstd[:], lnv[:], Act.Exp, scale=-0.5,
                                 bias=lnb[:, 2:3])
            w1 = z_pool.tile([P, D], f32, tag="w1")
            if t % 2 == 0:
                c1 = small.tile([P, 1], f32, tag="c1z")
                nc.vector.tensor_scalar(c1[:], agg[:, 0:1], rstd[:], -1.0,
                                        op0=Alu.mult, op1=Alu.mult)
                nc.scalar.activation(w1[:], z[:], Act.Identity,
                                     bias=c1[:], scale=rstd[:])
            else:
                nc.vector.tensor_scalar(w1[:], z[:], agg[:, 0:1], rstd[:],
                                        op0=Alu.subtract, op1=Alu.mult)
            if has_gobo:
                o_sb = z_pool.tile([P, D], f32, tag="osb")
                nc.vector.tensor_tensor(o_sb[:], w1[:], go_sb, op=Alu.mult)
                nc.vector.tensor_tensor(o_sb[:], o_sb[:], bo_sb, op=Alu.add)
                nc.scalar.dma_start(out_d[t * P:(t + 1) * P, :], o_sb[:])
            else:
                nc.scalar.dma_start(out_d[t * P:(t + 1) * P, :], w1[:])

        loop_cm = tc.For_i(0, loop_n, 1) if loop_n else None
        if loop_cm is not None:
            loop_cm.__enter__()
        # ---- stage 1 (q/k transposes lag their projections by ~3 groups
        #      so the LN chain is never on the PE's critical path) ----
        pending = []
        for i in range(NT):
            for nm, wkey, kind in (("xqT", "wq", "q"),
                                   ("xkT", "wk", "k"),
                                   ("xvT", "wv", "v")):
                if (nm, i) in prefetched:
                    xtile = prefetched.pop((nm, i))
                else:
                    xtile = xstage.tile([P, KC * P], bf16, tag=nm)
                    nc.sync.dma_start(xtile[:],
                                      xT_view[nm][i * P:(i + 1) * P, :])
                res = proj_tile(xtile[:], w_r[wkey], i, kind)
                if res is not None:
                    pending.append(res)
                while len(pending) > 2:
                    k2, i2, y2 = pending.pop(0)
                    emit_qk_transpose(k2, i2, y2, tp_half())
        for k2, i2, y2 in pending:
            emit_qk_transpose(k2, i2, y2, tp_half())
        pending = []
        # ---- stage 2 (PV lags S by one query tile) ----
        prev = None
        for t in range(NT):
            state = s_block(t)
            if prev is not None:
                pv_block(t - 1, prev)
            prev = state
        pv_block(NT - 1, prev)
        if loop_cm is not None:
            loop_cm.__exit__(None, None, None)

    nc.compile()
    return nc


def _get_nc(has_km=False, has_gobo=False):
    key = ("nc", bool(has_km), bool(has_gobo))
    if key not in _CACHE:
        _CACHE[key] = _build(has_km, has_gobo=has_gobo)
    return _CACHE[key]


def _bench_inputs(rng):
    """Input map (one core) with the same shapes/dtypes kernel() feeds."""
    f = np.float32
    bf = _bf16np()
    xq = rng.standard_normal((S, D), dtype=f)
    pack = lambda x: np.ascontiguousarray(
        x.reshape(NT, P, KC, P).transpose(0, 3, 2, 1)).reshape(S, D).astype(bf)
    mkT = lambda: pack(rng.standard_normal((S, D), dtype=f))
    lnb = np.zeros((P, 3), f)
    lnb[:, 0] = -0.5 * math.log(D)
    return {
        "xq": xq, "xqT": pack(xq),
        "xkT": mkT(), "xvT": mkT(),
        "wq": rng.standard_normal((D, D), dtype=f).astype(bf),
        "wk": rng.standard_normal((D, D), dtype=f).astype(bf),
        "wv": rng.standard_normal((D, D), dtype=f).astype(bf),
        "lnb": lnb, "gobo": np.ones((P, 2 * D), f),
        "km": np.zeros((P, S), f),
    }


def _fallback(vals, keys, ques, causal_mask, key_mask, Wv, Wk, Wq,
              ln_k_g, ln_k_b, ln_q_g, ln_q_b, ln_o_g, ln_o_b):
    # numpy reference path; used when the inputs fall outside the pattern
    # this kernel is specialized for.
    def ln(x, g, b):
        mu = x.mean(-1, keepdims=True)
        var = ((x - mu) ** 2).mean(-1, keepdims=True)
        return (x - mu) / np.sqrt(var + EPS) * g + b

    x64 = np.float64
    vals, keys, ques = (np.asarray(a) for a in (vals, keys, ques))
    v = vals.astype(x64) @ np.asarray(Wv, x64)
    k = ln(keys.astype(x64) @ np.asarray(Wk, x64), np.asarray(ln_k_g),
           np.asarray(ln_k_b))
    q = ln(ques.astype(x64) @ np.asarray(Wq, x64), np.asarray(ln_q_g),
           np.asarray(ln_q_b))
    a = np.einsum("bqd,bkd->bqk", q, k) / math.sqrt(D)
    a = np.where(causal_mask[None], -np.inf, a)
    a = np.where(key_mask[:, None, :], -np.inf, a)
    a = a - a.max(-1, keepdims=True)
    p = np.exp(a)
    p /= p.sum(-1, keepdims=True)
    o = np.einsum("bqk,bkd->bqd", p, v)
    return np.asarray(ln(o + ques.astype(x64), np.asarray(ln_o_g),
                         np.asarray(ln_o_b)), np.float32)


def _get_runner(has_km, has_gobo):
    """Build (once) a cached sharded-jit executor for the compiled module."""
    key = ("runner", bool(has_km), bool(has_gobo))
    if key in _CACHE:
        return _CACHE[key]

    import jax
    import numpy as _np
    from jax.sharding import Mesh, PartitionSpec
    from jax.experimental.shard_map import shard_map
    from concourse import mybir
    from concourse.bass2jax import (_bass_exec_p, install_neuronx_cc_hook,
                                    partition_id_tensor)

    install_neuronx_cc_hook()
    nc = _get_nc(has_km, has_gobo)

    pname = nc.partition_id_tensor.name if nc.partition_id_tensor else None
    in_names, out_names, out_avals, zero_outs = [], [], [], []
    for alloc in nc.m.functions[0].allocations:
        if not isinstance(alloc, mybir.MemoryLocationSet):
            continue
        name = alloc.memorylocations[0].name
        if alloc.kind == "ExternalInput":
            if name != pname:
                in_names.append(name)
        elif alloc.kind == "ExternalOutput":
            shape = tuple(alloc.tensor_shape)
            dtype = mybir.dt.np(alloc.dtype)
            out_names.append(name)
            out_avals.append(jax.core.ShapedArray(shape, dtype))
            zero_outs.append(_np.zeros((B * shape[0], *shape[1:]), dtype))
    n_params = len(in_names)
    all_in = in_names + out_names
    if pname is not None:
        all_in = all_in + [pname]

    def _body(*args):
        operands = list(args)
        if pname is not None:
            operands.append(partition_id_tensor())
        outs = _bass_exec_p.bind(
            *operands,
            out_avals=tuple(out_avals),
            in_names=tuple(all_in),
            out_names=tuple(out_names),
            lowering_input_output_aliases=(),
            sim_require_finite=True,
            sim_require_nnan=True,
            nc=nc,
        )
        return tuple(outs)

    devices = jax.devices()[:B]
    mesh = Mesh(np.asarray(devices), ("core",))
    donate = tuple(range(n_params, n_params + len(out_names)))
    sharded = jax.jit(
        shard_map(_body, mesh=mesh,
                  in_specs=(PartitionSpec("core"),) * (n_params + len(out_names)),
                  out_specs=(PartitionSpec("core"),) * len(out_names),
                  check_rep=False),
        donate_argnums=donate, keep_unused=True)

    def run(concat_by_name):
        args = [concat_by_name[n] for n in in_names] + list(zero_outs)
        out_arrs = sharded(*args)
        return {n: _np.asarray(out_arrs[i]).reshape(B, *out_avals[i].shape)
                for i, n in enumerate(out_names)}

    _CACHE[key] = run
    return run


def _uniform_pos(v):
    v = np.asarray(v, np.float32)
    return v.size > 0 and np.all(v == v.flat[0]) and v.flat[0] > 0


def kernel(vals, keys, ques, causal_mask, key_mask, Wv, Wk, Wq,
           ln_k_g, ln_k_b, ln_q_g, ln_q_b, ln_o_g, ln_o_b):
    causal_mask = np.asarray(causal_mask)
    key_mask = np.asarray(key_mask)
    ln_q_b = np.asarray(ln_q_b, np.float32)
    ln_k_b = np.asarray(ln_k_b, np.float32)
    ln_o_g = np.asarray(ln_o_g, np.float32)
    ln_o_b = np.asarray(ln_o_b, np.float32)
    # The device kernel is specialized for: standard causal triu mask,
    # b_q == 0 (b_k is dropped exactly -- it shifts each logit row by a
    # per-query constant, which softmax ignores), uniform positive q/k
    # gains (folded into the Exp bias producing rstd).
    ok = (np.array_equal(causal_mask, np.triu(np.ones((S, S), bool), k=1))
          and not ln_q_b.any()
          and float(np.abs(ln_k_b).max(initial=0.0)) < 16.0
          and _uniform_pos(ln_q_g) and _uniform_pos(ln_k_g))
    if not ok:
        return _fallback(vals, keys, ques, causal_mask, key_mask, Wv, Wk, Wq,
                         ln_k_g, ln_k_b, ln_q_g, ln_q_b, ln_o_g, ln_o_b)

    has_km = bool(key_mask.any())
    has_gobo = not (_uniform_pos(ln_o_g) and not ln_o_b.any())
    run = _get_runner(has_km, has_gobo)

    f = np.float32
    bf = _bf16np()

    lnb = np.zeros((P, 3), f)
    lnb[:, 0] = math.log(float(np.asarray(ln_q_g).flat[0]) / math.sqrt(D))
    lnb[:, 1] = math.log(float(np.asarray(ln_k_g).flat[0]))
    lnb[:, 2] = 0.0 if has_gobo else math.log(float(ln_o_g.flat[0]))
    gobo = np.broadcast_to(
        np.concatenate([ln_o_g, ln_o_b]), (P, 2 * D)).copy()
    xq = np.ascontiguousarray(ques, f).reshape(B * S, D)

    def xT(a):
        # [B, S, D] fp32 -> per-batch tile-major packed transpose, bf16:
        # row (i*P + p), col (c*P + s') = a[b, i*P + s', c*P + p]
        return np.ascontiguousarray(
            np.asarray(a, f).reshape(B, NT, P, KC, P).transpose(0, 1, 4, 3, 2)
        ).reshape(B * S, D).astype(bf)

    def rep(a):
        return np.concatenate([a] * B, axis=0)

    km_rows = np.where(key_mask, NEG, f(0)).astype(f)          # [B, S]
    km_cat = np.repeat(km_rows, P, axis=0)                      # [B*P, S]
    concat = {
        "xq": xq,
        "xqT": xT(ques), "xkT": xT(keys), "xvT": xT(vals),
        "wq": rep(np.ascontiguousarray(Wq, f).astype(bf)),
        "wk": rep(np.ascontiguousarray(Wk, f).astype(bf)),
        "wv": rep(np.ascontiguousarray(Wv, f).astype(bf)),
        "lnb": rep(lnb), "gobo": rep(gobo),
        "km": km_cat,
    }
    out = run(concat)["out"]                                    # [B, S, D]
    return out

